# revision 16
# baseline (speedup 1.0000x reference)
"""AttentiveMatchingLayer TRN2 kernel (v2: software-pipelined stages).

Math (per batch, validated against the jax reference):
  ssa[t] = sum_d a[t,d]^2 ; ssb likewise ; stok = 1/sqrt(ssa*ssb)
  as = a * stok[:,None]                     # carries BOTH l2 norms
  alpha[d,e] = sum_t b[t,d] * as[t,e]       # == ref alpha (norms folded)
  s_al[e] = 1/sqrt(sum_d alpha[d,e]^2)
  hmT[e,t] = sum_d alpha[d,e] * b[t,d]
     (differs from ref hmean by a per-token positive factor 1/rb[t],
      which cancels in the final cosine)
  num[t,p] = sum_d (a * hmT * s_al) W2 ; sa = sum_d a^2 W2
  sh[t,p]  = sum_d (hmT * s_al)^2 W2
  persp = num / sqrt(sa*sh + eps)
Sharding: data-parallel over batch B=32 across 8 cores (4 batches/core).

v2 structure: four stages per batch — A: transposes + row norms,
B: alpha + s_al, C: hmean + prod/hmsq (s_al folded in as a per-partition
scale: prod = (hp*sal) .* aT via scalar_tensor_tensor, hmsq =
Square(sal*hp) via activation scale), D: finals + divide + store.
Stages are emitted software-pipelined across the 4 batches so the PE
stream stays dense while loads arrive. ssb comes from one-pass DVE
square-with-accumulate; ssa rides a ones column on the sa matmul.
PSUM (8 banks): 3 transpose + 1 alpha + 2 hmean(+s_al) + 1 sa + 1 finals.
Output stored c-major so each partition writes one contiguous 640B run.
"""

import numpy as np
from contextlib import ExitStack

import concourse.bacc as bacc
import concourse.bass as bass
import concourse.tile as tile
from concourse import masks, mybir

B, T, D, P = 32, 1024, 256, 20
PA = P + 1         # w2t augmented with a ones column (-> ssa)
N_CORES = 8
NB = B // N_CORES  # batches per core
TC = T // 128      # 8 token chunks
DC = D // 128      # 2 d chunks
JC = TC // 2       # 4 (c = 2j + h)
F32 = mybir.dt.float32
F16 = mybir.dt.float16
EPS = 1e-12
Square = mybir.ActivationFunctionType.Square
Sqrt = mybir.ActivationFunctionType.Sqrt
Mult = mybir.AluOpType.mult



def build_kernel():
    nc = bacc.Bacc("TRN2", target_bir_lowering=False, debug=False,
                   num_devices=N_CORES)
    a_in = nc.declare_dram_parameter("a", [NB, T, D], F32, isOutput=False)
    b_in = nc.declare_dram_parameter("b", [NB, T, D], F32, isOutput=False)
    w2t_in = nc.declare_dram_parameter("w2t", [D, PA], F32, isOutput=False)
    out_d = nc.declare_dram_parameter("out", [NB, T, P], F32, isOutput=True)

    with tile.TileContext(nc) as tc, ExitStack() as ctx:
        consts = ctx.enter_context(tc.tile_pool(name="consts", bufs=1))
        p4 = ctx.enter_context(tc.tile_pool(name="p4", bufs=NB))
        pscr = ctx.enter_context(tc.tile_pool(name="pscr", bufs=2))
        ps_tr = ctx.enter_context(
            tc.tile_pool(name="ps_tr", bufs=3, space="PSUM"))
        ps_al = ctx.enter_context(
            tc.tile_pool(name="ps_al", bufs=1, space="PSUM"))
        ps_hp = ctx.enter_context(
            tc.tile_pool(name="ps_hp", bufs=2, space="PSUM"))
        ps_sa = ctx.enter_context(
            tc.tile_pool(name="ps_sa", bufs=1, space="PSUM"))
        ps_fin = ctx.enter_context(
            tc.tile_pool(name="ps_fin", bufs=1, space="PSUM"))

        # ---- input loads first: the DMA stream is the roofline ----
        w2t = consts.tile([128, DC, PA], F16)
        nc.gpsimd.dma_start(
            out=w2t[:], in_=w2t_in.ap().rearrange("(dc p) w -> p dc w", p=128))
        NBR = range(NB)
        a_sb, b_sb = [], []
        for b in NBR:
            a_sb.append(p4.tile([128, TC, D], F16, tag="a_sb", name=f"a_sb{b}"))
            b_sb.append(p4.tile([128, TC, D], F16, tag="b_sb", name=f"b_sb{b}"))
        nc.gpsimd.dma_start(
            out=a_sb[0][:], in_=a_in.ap()[0].rearrange("(p c) d -> p c d", p=128))
        nc.gpsimd.dma_start(
            out=b_sb[0][:], in_=b_in.ap()[0].rearrange("(p c) d -> p c d", p=128))

        identf = consts.tile([128, 128], F32)
        masks.make_identity(nc, identf[:])
        ident = consts.tile([128, 128], F16)
        nc.vector.tensor_copy(ident[:], identf[:])
        ones = consts.tile([128, 1], F16)
        nc.vector.memset(ones[:], 1.0)
        eps_sb = consts.tile([128, 1], F32)
        nc.vector.memset(eps_sb[:], EPS)
        # warm the ACT tables (Square/Sqrt) while DMAs stream
        actscr = consts.tile([128, 1], F32)
        nc.scalar.activation(actscr[:], eps_sb[:], Square)
        nc.scalar.activation(actscr[:], eps_sb[:], Sqrt)

        for b in range(1, NB):
            nc.gpsimd.dma_start(
                out=a_sb[b][:],
                in_=a_in.ap()[b].rearrange("(p c) d -> p c d", p=128))
            nc.gpsimd.dma_start(
                out=b_sb[b][:],
                in_=b_in.ap()[b].rearrange("(p c) d -> p c d", p=128))

        # ---- per-batch SBUF tiles ----
        aT_sb, bT_sb, asq_sb = [], [], []
        as_sb, ssb, stok, sa_sb, sal = [], [], [], [], []
        alpha_sb, alsq_sb, prod_sb, hmsq_sb = [], [], [], []
        for b in NBR:
            aT_sb.append(p4.tile([128, DC, T], F16, tag="aT", name=f"aT{b}"))
            bT_sb.append(p4.tile([128, DC, T], F16, tag="bT", name=f"bT{b}"))
            asq_sb.append(p4.tile([128, DC, T], F16, tag="asq", name=f"asq{b}"))
            as_sb.append(p4.tile([128, TC, D], F16, tag="as", name=f"as{b}"))
            ssb.append(p4.tile([128, TC], F32, tag="ssb", name=f"ssb{b}"))
            stok.append(p4.tile([128, TC], F32, tag="stok", name=f"stok{b}"))
            sa_sb.append(p4.tile([128, 2, JC * PA], F32, tag="sa",
                                 name=f"sa{b}"))
            sal.append(p4.tile([128, 2], F32, tag="sal", name=f"sal{b}"))
            alpha_sb.append(p4.tile([128, DC, 256], F16, tag="alpha",
                                    name=f"alpha{b}"))
            alsq_sb.append(p4.tile([128, DC, 256], F16, tag="alsq",
                                   name=f"alsq{b}"))
            prod_sb.append(p4.tile([128, DC, T], F16, tag="prod",
                                   name=f"prod{b}"))
            hmsq_sb.append(p4.tile([128, DC, T], F16, tag="hmsq",
                                   name=f"hmsq{b}"))

        def stage_a(b):
            aT_ps = []
            for dc in range(DC):
                tp = ps_tr.tile([128, T], F16, tag="tr", name=f"aTp{b}_{dc}")
                for c in range(TC):
                    nc.tensor.transpose(
                        out=tp[:, c * 128:(c + 1) * 128],
                        in_=a_sb[b][:, c, dc * 128:(dc + 1) * 128],
                        identity=ident[:])
                aT_ps.append(tp)
            for dc in range(DC):
                nc.vector.tensor_copy(aT_sb[b][:, dc, :], aT_ps[dc][:])
                nc.scalar.activation(asq_sb[b][:, dc, :], aT_ps[dc][:], Square)
            for dc in range(DC):
                tp = ps_tr.tile([128, T], F16, tag="tr", name=f"bTp{b}_{dc}")
                for c in range(TC):
                    nc.tensor.transpose(
                        out=tp[:, c * 128:(c + 1) * 128],
                        in_=b_sb[b][:, c, dc * 128:(dc + 1) * 128],
                        identity=ident[:])
                nc.vector.tensor_copy(bT_sb[b][:, dc, :], tp[:])
            # sa matmuls into a dedicated bank (ones col -> ssa)
            sm = ps_sa.tile([128, 512], F32, tag="sa", name=f"smsa{b}")
            for c in range(TC):
                off = (c % 2) * (JC * PA) + (c // 2) * PA
                for dc in range(DC):
                    nc.tensor.matmul(
                        sm[:, off:off + PA],
                        lhsT=asq_sb[b][:, dc, c * 128:(c + 1) * 128],
                        rhs=w2t[:, dc, :],
                        start=(dc == 0), stop=(dc == DC - 1))
            for h in range(2):
                nc.vector.tensor_copy(
                    sa_sb[b][:, h, :], sm[:, h * (JC * PA):(h + 1) * (JC * PA)])
            # ssb: square + row-reduce in one DVE pass per chunk
            for c in range(TC):
                if c % 2 == 0:
                    sq = pscr.tile([128, D], F16, tag="sqa", name=f"sqa{b}_{c}")
                    nc.scalar.activation(
                        sq[:], b_sb[b][:, c, :], Square,
                        accum_out=ssb[b][:, c:c + 1])
                else:
                    sq = pscr.tile([128, D], F16, tag="sq", name=f"sq{b}_{c}")
                    nc.vector.scalar_tensor_tensor(
                        out=sq[:], in0=b_sb[b][:, c, :], scalar=1.0,
                        in1=b_sb[b][:, c, :], op0=Mult, op1=Mult,
                        accum_out=ssb[b][:, c:c + 1])
            # stok = rsqrt(ssa*ssb)
            st = stok[b]
            nc.vector.tensor_mul(
                st[:].rearrange("q (j h) -> q h j", h=2),
                sa_sb[b][:].rearrange("q h (j w) -> q h j w", w=PA)[:, :, :, P],
                ssb[b][:].rearrange("q (j h) -> q h j", h=2))
            nc.scalar.activation(st[:], st[:], Sqrt)
            nc.vector.reciprocal(st[:], st[:])
            # as = a * stok (one DVE pass; stok broadcast over d)
            nc.vector.tensor_tensor(
                as_sb[b][:], a_sb[b][:],
                st[:].broadcast_to((128, TC, D)),
                op=Mult)

        def stage_b(b):
            # dc-outer: one accumulation group open per PSUM bank at a time
            # (interleaved open groups in one bank corrupt has_written bits)
            al = ps_al.tile([128, DC, 256], F32, tag="al", name=f"al{b}")
            for dc in range(DC):
                for c in range(TC):
                    nc.tensor.matmul(
                        al[:, dc, :],
                        lhsT=b_sb[b][:, c, dc * 128:(dc + 1) * 128],
                        rhs=as_sb[b][:, c, :],
                        start=(c == 0), stop=(c == TC - 1))
            for dc in range(DC):
                nc.scalar.copy(alpha_sb[b][:, dc, :], al[:, dc, :])
                nc.scalar.activation(alsq_sb[b][:, dc, :], al[:, dc, :], Square)
            salp = ps_hp.tile([128, 2], F32, tag="hp", name=f"salp{b}")
            for ec in range(2):
                for dc in range(DC):
                    nc.tensor.matmul(
                        salp[:, ec:ec + 1],
                        lhsT=alsq_sb[b][:, dc, ec * 128:(ec + 1) * 128],
                        rhs=ones[:],
                        start=(dc == 0), stop=(dc == DC - 1))
            nc.scalar.activation(sal[b][:], salp[:], Sqrt)
            nc.vector.reciprocal(sal[b][:], sal[b][:])

        def stage_c(b):
            for ec in range(2):
                for t2 in range(2):
                    hp = ps_hp.tile([128, 512], F32, tag="hp",
                                    name=f"hp{b}_{ec}_{t2}")
                    for dc in range(DC):
                        nc.tensor.matmul(
                            hp[:],
                            lhsT=alpha_sb[b][:, dc, ec * 128:(ec + 1) * 128],
                            rhs=bT_sb[b][:, dc, t2 * 512:(t2 + 1) * 512],
                            start=(dc == 0), stop=(dc == DC - 1))
                    sl = slice(t2 * 512, t2 * 512 + 512)
                    # prod = (hp*sal) .* aT  (s_al folded in here)
                    nc.vector.scalar_tensor_tensor(
                        out=prod_sb[b][:, ec, sl], in0=hp[:],
                        scalar=sal[b][:, ec:ec + 1],
                        in1=aT_sb[b][:, ec, sl], op0=Mult, op1=Mult)
                    # hmsq = Square(sal*hp)
                    nc.scalar.activation(
                        hmsq_sb[b][:, ec, sl], hp[:], Square,
                        scale=sal[b][:, ec:ec + 1])

        def stage_d(b):
            sm = ps_fin.tile([128, 512], F32, tag="fin", name=f"fin{b}")
            for src, base in ((prod_sb[b], 0), (hmsq_sb[b], 2 * JC * P)):
                for c in range(TC):
                    off = base + (c % 2) * (JC * P) + (c // 2) * P
                    for dc in range(DC):
                        nc.tensor.matmul(
                            sm[:, off:off + P],
                            lhsT=src[:, dc, c * 128:(c + 1) * 128],
                            rhs=w2t[:, dc, 0:P],
                            start=(dc == 0), stop=(dc == DC - 1))
            den = pscr.tile([128, 2, JC, P], F32, tag="den", name=f"den{b}")
            persp = pscr.tile([128, JC, 2, P], F32, tag="persp",
                              name=f"persp{b}")
            nc.vector.tensor_mul(
                den[:],
                sa_sb[b][:].rearrange(
                    "q h (j w) -> q h j w", w=PA)[:, :, :, 0:P],
                sm[:, 2 * (JC * P):4 * (JC * P)]
                .rearrange("q (h j w) -> q h j w", h=2, w=P))
            nc.scalar.activation(
                den[:].rearrange("q h j w -> q (h j w)"),
                den[:].rearrange("q h j w -> q (h j w)"), Sqrt,
                bias=eps_sb[:])
            nc.vector.reciprocal(
                den[:].rearrange("q h j w -> q (h j w)"),
                den[:].rearrange("q h j w -> q (h j w)"))
            nc.vector.tensor_mul(
                persp[:].rearrange("q j h w -> q h j w"),
                sm[:, 0:2 * (JC * P)]
                .rearrange("q (h j w) -> q h j w", h=2, w=P),
                den[:])
            nc.sync.dma_start(
                out=out_d.ap()[b].rearrange("(q j h) w -> q j h w", j=JC, h=2),
                in_=persp[:])

        # software-pipelined emission
        stage_a(0); stage_a(1); stage_b(0); stage_c(0); stage_b(1)
        stage_d(0); stage_a(2); stage_c(1); stage_b(2); stage_d(1)
        stage_a(3); stage_c(2); stage_b(3); stage_d(2); stage_c(3)
        stage_d(3)

    nc.compile()
    return nc


_NC_CACHE = None


def _get_nc():
    global _NC_CACHE
    if _NC_CACHE is None:
        _NC_CACHE = build_kernel()
    return _NC_CACHE


def _make_in_maps(inp_a, inp_b, W):
    inp_a = np.ascontiguousarray(np.asarray(inp_a, dtype=np.float32))
    inp_b = np.ascontiguousarray(np.asarray(inp_b, dtype=np.float32))
    W = np.asarray(W, dtype=np.float32)
    w2t = np.ones((D, PA), dtype=np.float32)
    w2t[:, :P] = (W * W).T
    return [
        {"a": inp_a[k * NB:(k + 1) * NB], "b": inp_b[k * NB:(k + 1) * NB],
         "w2t": w2t}
        for k in range(N_CORES)
    ]


def kernel(inp_a, inp_b, W):
    from concourse.bass_utils import run_bass_kernel_spmd
    nc = _get_nc()
    in_maps = _make_in_maps(inp_a, inp_b, W)
    res = run_bass_kernel_spmd(nc, in_maps, list(range(N_CORES)))
    persp = np.concatenate(
        [res.results[k]["out"] for k in range(N_CORES)], axis=0)
    return (persp, persp)


if __name__ == "__main__":
    rng = np.random.default_rng(0)
    inputs = {
        "inp_a": rng.standard_normal((B, T, D), dtype=np.float32),
        "inp_b": rng.standard_normal((B, T, D), dtype=np.float32),
        "W": rng.uniform(-0.05, 0.05, (P, D)).astype(np.float32),
    }
    out = kernel(**inputs)
    print("ok", out[0].shape, out[0].dtype)


# revision 17
# speedup vs baseline: 1.0065x; 1.0065x over previous
"""AttentiveMatchingLayer TRN2 kernel (v2: software-pipelined stages).

Math (per batch, validated against the jax reference):
  ssa[t] = sum_d a[t,d]^2 ; ssb likewise ; stok = 1/sqrt(ssa*ssb)
  as = a * stok[:,None]                     # carries BOTH l2 norms
  alpha[d,e] = sum_t b[t,d] * as[t,e]       # == ref alpha (norms folded)
  s_al[e] = 1/sqrt(sum_d alpha[d,e]^2)
  hmT[e,t] = sum_d alpha[d,e] * b[t,d]
     (differs from ref hmean by a per-token positive factor 1/rb[t],
      which cancels in the final cosine)
  num[t,p] = sum_d (a * hmT * s_al) W2 ; sa = sum_d a^2 W2
  sh[t,p]  = sum_d (hmT * s_al)^2 W2
  persp = num / sqrt(sa*sh + eps)
Sharding: data-parallel over batch B=32 across 8 cores (4 batches/core).

v2 structure: four stages per batch — A: transposes + row norms,
B: alpha + s_al, C: hmean + prod/hmsq (s_al folded in as a per-partition
scale: prod = (hp*sal) .* aT via scalar_tensor_tensor, hmsq =
Square(sal*hp) via activation scale), D: finals + divide + store.
Stages are emitted software-pipelined across the 4 batches so the PE
stream stays dense while loads arrive. ssb comes from one-pass DVE
square-with-accumulate; ssa rides a ones column on the sa matmul.
PSUM (8 banks): 3 transpose + 1 alpha + 2 hmean(+s_al) + 1 sa + 1 finals.
Output stored c-major so each partition writes one contiguous 640B run.
"""

import numpy as np
from contextlib import ExitStack

import concourse.bacc as bacc
import concourse.bass as bass
import concourse.tile as tile
from concourse import masks, mybir

B, T, D, P = 32, 1024, 256, 20
PA = P + 1         # w2t augmented with a ones column (-> ssa)
N_CORES = 8
NB = B // N_CORES  # batches per core
TC = T // 128      # 8 token chunks
DC = D // 128      # 2 d chunks
JC = TC // 2       # 4 (c = 2j + h)
F32 = mybir.dt.float32
F16 = mybir.dt.float16
EPS = 1e-12
Square = mybir.ActivationFunctionType.Square
Sqrt = mybir.ActivationFunctionType.Sqrt
Mult = mybir.AluOpType.mult



def build_kernel():
    nc = bacc.Bacc("TRN2", target_bir_lowering=False, debug=False,
                   num_devices=N_CORES)
    a_in = nc.declare_dram_parameter("a", [NB, T, D], F32, isOutput=False)
    b_in = nc.declare_dram_parameter("b", [NB, T, D], F32, isOutput=False)
    w2t_in = nc.declare_dram_parameter("w2t", [D, PA], F32, isOutput=False)
    out_d = nc.declare_dram_parameter("out", [NB, T, P], F32, isOutput=True)

    with tile.TileContext(nc) as tc, ExitStack() as ctx:
        consts = ctx.enter_context(tc.tile_pool(name="consts", bufs=1))
        p4 = ctx.enter_context(tc.tile_pool(name="p4", bufs=NB))
        pscr = ctx.enter_context(tc.tile_pool(name="pscr", bufs=2))
        ps_tr = ctx.enter_context(
            tc.tile_pool(name="ps_tr", bufs=3, space="PSUM"))
        ps_al = ctx.enter_context(
            tc.tile_pool(name="ps_al", bufs=1, space="PSUM"))
        ps_hp = ctx.enter_context(
            tc.tile_pool(name="ps_hp", bufs=2, space="PSUM"))
        ps_sa = ctx.enter_context(
            tc.tile_pool(name="ps_sa", bufs=1, space="PSUM"))
        ps_fin = ctx.enter_context(
            tc.tile_pool(name="ps_fin", bufs=1, space="PSUM"))

        # ---- input loads first: the DMA stream is the roofline ----
        NBR = range(NB)
        a_sb, b_sb = [], []
        for b in NBR:
            a_sb.append(p4.tile([128, TC, D], F16, tag="a_sb", name=f"a_sb{b}"))
            b_sb.append(p4.tile([128, TC, D], F16, tag="b_sb", name=f"b_sb{b}"))
        nc.gpsimd.dma_start(
            out=a_sb[0][:], in_=a_in.ap()[0].rearrange("(p c) d -> p c d", p=128))
        nc.gpsimd.dma_start(
            out=b_sb[0][:], in_=b_in.ap()[0].rearrange("(p c) d -> p c d", p=128))
        w2t = consts.tile([128, DC, PA], F16)
        nc.gpsimd.dma_start(
            out=w2t[:], in_=w2t_in.ap().rearrange("(dc p) w -> p dc w", p=128))

        identf = consts.tile([128, 128], F32)
        masks.make_identity(nc, identf[:])
        ident = consts.tile([128, 128], F16)
        nc.vector.tensor_copy(ident[:], identf[:])
        ones = consts.tile([128, 1], F16)
        nc.vector.memset(ones[:], 1.0)
        eps_sb = consts.tile([128, 1], F32)
        nc.vector.memset(eps_sb[:], EPS)
        # warm the ACT tables (Square/Sqrt) while DMAs stream
        actscr = consts.tile([128, 1], F32)
        nc.scalar.activation(actscr[:], eps_sb[:], Square)
        nc.scalar.activation(actscr[:], eps_sb[:], Sqrt)

        for b in range(1, NB):
            nc.gpsimd.dma_start(
                out=a_sb[b][:],
                in_=a_in.ap()[b].rearrange("(p c) d -> p c d", p=128))
            nc.gpsimd.dma_start(
                out=b_sb[b][:],
                in_=b_in.ap()[b].rearrange("(p c) d -> p c d", p=128))

        # ---- per-batch SBUF tiles ----
        aT_sb, bT_sb, asq_sb = [], [], []
        as_sb, ssb, stok, sa_sb, sal = [], [], [], [], []
        alpha_sb, alsq_sb, prod_sb, hmsq_sb = [], [], [], []
        for b in NBR:
            aT_sb.append(p4.tile([128, DC, T], F16, tag="aT", name=f"aT{b}"))
            bT_sb.append(p4.tile([128, DC, T], F16, tag="bT", name=f"bT{b}"))
            asq_sb.append(p4.tile([128, DC, T], F16, tag="asq", name=f"asq{b}"))
            as_sb.append(p4.tile([128, TC, D], F16, tag="as", name=f"as{b}"))
            ssb.append(p4.tile([128, TC], F32, tag="ssb", name=f"ssb{b}"))
            stok.append(p4.tile([128, TC], F32, tag="stok", name=f"stok{b}"))
            sa_sb.append(p4.tile([128, 2, JC * PA], F32, tag="sa",
                                 name=f"sa{b}"))
            sal.append(p4.tile([128, 2], F32, tag="sal", name=f"sal{b}"))
            alpha_sb.append(p4.tile([128, DC, 256], F16, tag="alpha",
                                    name=f"alpha{b}"))
            alsq_sb.append(p4.tile([128, DC, 256], F16, tag="alsq",
                                   name=f"alsq{b}"))
            prod_sb.append(p4.tile([128, DC, T], F16, tag="prod",
                                   name=f"prod{b}"))
            hmsq_sb.append(p4.tile([128, DC, T], F16, tag="hmsq",
                                   name=f"hmsq{b}"))

        def stage_a(b):
            aT_ps = []
            for dc in range(DC):
                tp = ps_tr.tile([128, T], F16, tag="tr", name=f"aTp{b}_{dc}")
                for c in range(TC):
                    nc.tensor.transpose(
                        out=tp[:, c * 128:(c + 1) * 128],
                        in_=a_sb[b][:, c, dc * 128:(dc + 1) * 128],
                        identity=ident[:])
                aT_ps.append(tp)
            for dc in range(DC):
                nc.vector.tensor_copy(aT_sb[b][:, dc, :], aT_ps[dc][:])
                nc.scalar.activation(asq_sb[b][:, dc, :], aT_ps[dc][:], Square)
            for dc in range(DC):
                tp = ps_tr.tile([128, T], F16, tag="tr", name=f"bTp{b}_{dc}")
                for c in range(TC):
                    nc.tensor.transpose(
                        out=tp[:, c * 128:(c + 1) * 128],
                        in_=b_sb[b][:, c, dc * 128:(dc + 1) * 128],
                        identity=ident[:])
                nc.vector.tensor_copy(bT_sb[b][:, dc, :], tp[:])
            # sa matmuls into a dedicated bank (ones col -> ssa)
            sm = ps_sa.tile([128, 512], F32, tag="sa", name=f"smsa{b}")
            for c in range(TC):
                off = (c % 2) * (JC * PA) + (c // 2) * PA
                for dc in range(DC):
                    nc.tensor.matmul(
                        sm[:, off:off + PA],
                        lhsT=asq_sb[b][:, dc, c * 128:(c + 1) * 128],
                        rhs=w2t[:, dc, :],
                        start=(dc == 0), stop=(dc == DC - 1))
            for h in range(2):
                nc.vector.tensor_copy(
                    sa_sb[b][:, h, :], sm[:, h * (JC * PA):(h + 1) * (JC * PA)])
            # ssb: square + row-reduce in one DVE pass per chunk
            for c in range(TC):
                if c % 2 == 0:
                    sq = pscr.tile([128, D], F16, tag="sqa", name=f"sqa{b}_{c}")
                    nc.scalar.activation(
                        sq[:], b_sb[b][:, c, :], Square,
                        accum_out=ssb[b][:, c:c + 1])
                else:
                    sq = pscr.tile([128, D], F16, tag="sq", name=f"sq{b}_{c}")
                    nc.vector.scalar_tensor_tensor(
                        out=sq[:], in0=b_sb[b][:, c, :], scalar=1.0,
                        in1=b_sb[b][:, c, :], op0=Mult, op1=Mult,
                        accum_out=ssb[b][:, c:c + 1])
            # stok = rsqrt(ssa*ssb)
            st = stok[b]
            nc.vector.tensor_mul(
                st[:].rearrange("q (j h) -> q h j", h=2),
                sa_sb[b][:].rearrange("q h (j w) -> q h j w", w=PA)[:, :, :, P],
                ssb[b][:].rearrange("q (j h) -> q h j", h=2))
            nc.scalar.activation(st[:], st[:], Sqrt)
            nc.vector.reciprocal(st[:], st[:])
            # as = a * stok (one DVE pass; stok broadcast over d)
            nc.vector.tensor_tensor(
                as_sb[b][:], a_sb[b][:],
                st[:].broadcast_to((128, TC, D)),
                op=Mult)

        def stage_b(b):
            # dc-outer: one accumulation group open per PSUM bank at a time
            # (interleaved open groups in one bank corrupt has_written bits)
            al = ps_al.tile([128, DC, 256], F32, tag="al", name=f"al{b}")
            for dc in range(DC):
                for c in range(TC):
                    nc.tensor.matmul(
                        al[:, dc, :],
                        lhsT=b_sb[b][:, c, dc * 128:(dc + 1) * 128],
                        rhs=as_sb[b][:, c, :],
                        start=(c == 0), stop=(c == TC - 1))
            for dc in range(DC):
                nc.scalar.copy(alpha_sb[b][:, dc, :], al[:, dc, :])
                nc.scalar.activation(alsq_sb[b][:, dc, :], al[:, dc, :], Square)
            salp = ps_hp.tile([128, 2], F32, tag="hp", name=f"salp{b}")
            for ec in range(2):
                for dc in range(DC):
                    nc.tensor.matmul(
                        salp[:, ec:ec + 1],
                        lhsT=alsq_sb[b][:, dc, ec * 128:(ec + 1) * 128],
                        rhs=ones[:],
                        start=(dc == 0), stop=(dc == DC - 1))
            nc.scalar.activation(sal[b][:], salp[:], Sqrt)
            nc.vector.reciprocal(sal[b][:], sal[b][:])

        def stage_c(b):
            for ec in range(2):
                for t2 in range(2):
                    hp = ps_hp.tile([128, 512], F32, tag="hp",
                                    name=f"hp{b}_{ec}_{t2}")
                    for dc in range(DC):
                        nc.tensor.matmul(
                            hp[:],
                            lhsT=alpha_sb[b][:, dc, ec * 128:(ec + 1) * 128],
                            rhs=bT_sb[b][:, dc, t2 * 512:(t2 + 1) * 512],
                            start=(dc == 0), stop=(dc == DC - 1))
                    sl = slice(t2 * 512, t2 * 512 + 512)
                    # prod = (hp*sal) .* aT  (s_al folded in here)
                    nc.vector.scalar_tensor_tensor(
                        out=prod_sb[b][:, ec, sl], in0=hp[:],
                        scalar=sal[b][:, ec:ec + 1],
                        in1=aT_sb[b][:, ec, sl], op0=Mult, op1=Mult)
                    # hmsq = Square(sal*hp)
                    nc.scalar.activation(
                        hmsq_sb[b][:, ec, sl], hp[:], Square,
                        scale=sal[b][:, ec:ec + 1])

        def stage_d(b):
            sm = ps_fin.tile([128, 512], F32, tag="fin", name=f"fin{b}")
            for src, base in ((prod_sb[b], 0), (hmsq_sb[b], 2 * JC * P)):
                for c in range(TC):
                    off = base + (c % 2) * (JC * P) + (c // 2) * P
                    for dc in range(DC):
                        nc.tensor.matmul(
                            sm[:, off:off + P],
                            lhsT=src[:, dc, c * 128:(c + 1) * 128],
                            rhs=w2t[:, dc, 0:P],
                            start=(dc == 0), stop=(dc == DC - 1))
            den = pscr.tile([128, 2, JC, P], F32, tag="den", name=f"den{b}")
            persp = pscr.tile([128, JC, 2, P], F32, tag="persp",
                              name=f"persp{b}")
            nc.vector.tensor_mul(
                den[:],
                sa_sb[b][:].rearrange(
                    "q h (j w) -> q h j w", w=PA)[:, :, :, 0:P],
                sm[:, 2 * (JC * P):4 * (JC * P)]
                .rearrange("q (h j w) -> q h j w", h=2, w=P))
            nc.scalar.activation(
                den[:].rearrange("q h j w -> q (h j w)"),
                den[:].rearrange("q h j w -> q (h j w)"), Sqrt,
                bias=eps_sb[:])
            nc.vector.reciprocal(
                den[:].rearrange("q h j w -> q (h j w)"),
                den[:].rearrange("q h j w -> q (h j w)"))
            nc.vector.tensor_mul(
                persp[:].rearrange("q j h w -> q h j w"),
                sm[:, 0:2 * (JC * P)]
                .rearrange("q (h j w) -> q h j w", h=2, w=P),
                den[:])
            nc.sync.dma_start(
                out=out_d.ap()[b].rearrange("(q j h) w -> q j h w", j=JC, h=2),
                in_=persp[:])

        # software-pipelined emission
        stage_a(0); stage_a(1); stage_b(0); stage_c(0); stage_b(1)
        stage_d(0); stage_a(2); stage_c(1); stage_b(2); stage_d(1)
        stage_a(3); stage_c(2); stage_b(3); stage_d(2); stage_c(3)
        stage_d(3)

    nc.compile()
    return nc


_NC_CACHE = None


def _get_nc():
    global _NC_CACHE
    if _NC_CACHE is None:
        _NC_CACHE = build_kernel()
    return _NC_CACHE


def _make_in_maps(inp_a, inp_b, W):
    inp_a = np.ascontiguousarray(np.asarray(inp_a, dtype=np.float32))
    inp_b = np.ascontiguousarray(np.asarray(inp_b, dtype=np.float32))
    W = np.asarray(W, dtype=np.float32)
    w2t = np.ones((D, PA), dtype=np.float32)
    w2t[:, :P] = (W * W).T
    return [
        {"a": inp_a[k * NB:(k + 1) * NB], "b": inp_b[k * NB:(k + 1) * NB],
         "w2t": w2t}
        for k in range(N_CORES)
    ]


def kernel(inp_a, inp_b, W):
    from concourse.bass_utils import run_bass_kernel_spmd
    nc = _get_nc()
    in_maps = _make_in_maps(inp_a, inp_b, W)
    res = run_bass_kernel_spmd(nc, in_maps, list(range(N_CORES)))
    persp = np.concatenate(
        [res.results[k]["out"] for k in range(N_CORES)], axis=0)
    return (persp, persp)


if __name__ == "__main__":
    rng = np.random.default_rng(0)
    inputs = {
        "inp_a": rng.standard_normal((B, T, D), dtype=np.float32),
        "inp_b": rng.standard_normal((B, T, D), dtype=np.float32),
        "W": rng.uniform(-0.05, 0.05, (P, D)).astype(np.float32),
    }
    out = kernel(**inputs)
    print("ok", out[0].shape, out[0].dtype)
